# revision 1
# baseline (speedup 1.0000x reference)
"""AttentionAugmentation2D Trainium2 kernel.

Shapes (hardcoded): B=8, H=W=32, N=1024, NH=8 heads, dk=dv=32 per head.
inputs [8,32,32,768] = q|k|v (256 each), key_rel_h/w [63,32].

Sharding: data-parallel over batch B across the 8 cores. Each core runs the
full 8-head attention for its batch.

Math per (batch, head), with n=(i,j), m=(i',j') (i = H index):
  logits[n,m] = qs[n]@k[m] + qs[(j,i)]@rel_h[i'-i+31] + qs[(i,j)]@rel_w[i'-i+31]
Both rel terms depend on m only through i', so with
  SWT[u,n] = rel_w[u]@qs[(i,j)] + rel_h[u]@qs[(j,i)]      (u in [0,63))
  biasT[t,n] = SWT[t+31-i(n), n]                          (shifted windows)
we get  logits^T = K_aug^T.T @ Q_augT  with contraction 64:
  K_aug^T rows: 0:32 = k^T, 32:64 = onehot[t==i'(m)]
  Q_augT rows:  0:32 = qs^T, 32:64 = biasT
Softmax without max-subtraction (logits bounded ~+-8 for randn inputs);
row sums come free from a ones-column appended to V in the attn@V matmul.

Toolchain note: walrus codegen only fits ONE semaphore wait in most TPB
instruction structs and does not split excess waits itself (stock kernels
trip this too).  split_multiwaits() below is a BIR post-pass that moves
excess waits onto same-engine InstNoOp carriers placed immediately before
the offending instruction — semantically identical, compiles everywhere.
"""

import numpy as np

import concourse.bass as bass
import concourse.mybir as mybir
import concourse.tile as tile
from concourse import bass_utils
from concourse.masks import make_identity

F32 = mybir.dt.float32
F32R = mybir.dt.float32r
AF = mybir.ActivationFunctionType

NH = 8
N = 1024
DK = 32
SCALE = float(DK) ** -0.5


def split_multiwaits(nc, dma_limit=1):
    """Move excess semaphore waits onto same-engine nop carriers."""
    n_new = 0
    for f in nc.m.functions:
        for blk in f.blocks:
            newlist = []
            for inst in blk.instructions:
                si = getattr(inst, "sync_info", None)
                is_dma = isinstance(inst, mybir.InstDMACopy)
                limit = dma_limit if is_dma else 1
                if si is not None and len(si.on_wait) > limit:
                    waits = list(si.on_wait)
                    for w in waits[:-1]:
                        n_new += 1
                        newlist.append(mybir.InstNoOp(
                            name=f"I-wc{n_new}",
                            ins=[], outs=[],
                            sync_info=mybir.SyncInfo(on_wait=[w], on_update=[]),
                            bass_nofuse=True,
                            engine=inst.engine,
                        ))
                    inst.sync_info = mybir.SyncInfo(
                        on_wait=waits[-1:], on_update=si.on_update)
                newlist.append(inst)
            blk.instructions = newlist
    return n_new


def kernel_body(tc, outs, ins):
    nc = tc.nc
    x = ins["x"]          # [1024, 768] rows n=(i,j), cols q|k|v
    relh = ins["relh"]    # [63, 32]
    relw = ins["relw"]    # [63, 32]
    out = outs["out"]     # [1024, 256]

    with (
        tc.tile_pool(name="persist", bufs=1) as persist,
        tc.tile_pool(name="expw", bufs=4) as expwp,
        tc.tile_pool(name="stage", bufs=2) as stagep,
        tc.tile_pool(name="dram", bufs=1, space="DRAM") as dramp,
        tc.tile_pool(name="psum_log", bufs=2, space="PSUM") as pslog,
        tc.tile_pool(name="psum_sw", bufs=1, space="PSUM") as pssw,
        tc.tile_pool(name="psum_att", bufs=1, space="PSUM") as psatt,
    ):
        # ---------------- startup constants ----------------
        ident = persist.tile([128, 128], F32)
        make_identity(nc, ident)
        ident_marker = nc.gpsimd.tensor_copy(ident[0:1, 0:1], ident[0:1, 0:1])

        rows_all = persist.tile([128, 8, 512], F32)
        for rh in range(4):
            rows_src = bass.AP(
                tensor=x.tensor, offset=rh * 2 * 128 * 768,
                ap=[[768, 128], [128 * 768, 2], [1, 512]])
            nc.sync.dma_start(out=rows_all[:, rh * 2:(rh + 1) * 2, :], in_=rows_src)

        rel_st = persist.tile([64, 63], F32R)
        nc.sync.dma_start(out=rel_st[0:32], in_=relw.rearrange("u d -> d u").bitcast(F32R))
        nc.sync.dma_start(out=rel_st[32:64], in_=relh.rearrange("u d -> d u").bitcast(F32R))

        # v with ones column appended: v_aug[p, h, chunk, 0:32]=v, [...,32]=1
        # (constants staged in f32, DVE-copied so the write is f32r-"rounded"
        # as the BIR verifier requires for f32r matmul operands)
        v_aug = persist.tile([128, NH, 8, 33], F32R)
        ones_st = persist.tile([128, 64], F32)
        nc.gpsimd.memset(ones_st, 1.0)
        nc.vector.tensor_copy(
            v_aug[:, :, :, 32:33].rearrange("p h j o -> p (h j o)"), ones_st)
        for j in range(8):
            nc.sync.dma_start(
                out=v_aug[:, :, j, 0:32],
                in_=x[j * 128:(j + 1) * 128, 512:768].rearrange(
                    "p (h d) -> p h d", h=NH).bitcast(F32R),
            )

        # K_aug per-head tensors [64, mtile, 128]; rows 32:64 = onehot const
        ka = [persist.tile([64, 8, 128], F32R, tag=f"ka{i}", name=f"ka{i}")
              for i in range(4)]
        oh_st = persist.tile([32, 8, 128], F32)
        nc.gpsimd.memset(oh_st, 0.0)
        oh = oh_st.rearrange("t j (b m) -> t j b m", b=4)
        # fill 1.0 where partition t == 4j + b (relative partition idx)
        nc.gpsimd.affine_select(
            out=oh, in_=oh, compare_op=mybir.AluOpType.not_equal,
            fill=1.0, base=0, pattern=[[-4, 8], [-1, 4], [0, 32]],
            channel_multiplier=1)
        from concourse.tile import add_dep_helper
        for t in ka:
            cp = nc.gpsimd.tensor_copy(t[32:64], oh_st)
            add_dep_helper(cp.ins, ident_marker.ins, sync=False,
                           reason="ident first on Pool")

        # ---------------- input transposes ----------------
        # qT_[0]: heads 0-3 (partition = 32*(h%4)+d), qT_[1]: heads 4-7; same k.
        qT = [persist.tile([128, N], F32R, tag=f"qT{i}", name=f"qT{i}")
              for i in range(2)]
        kT = [persist.tile([128, N], F32R, tag=f"kT{i}", name=f"kT{i}")
              for i in range(2)]
        # type-major order: all q-half0 transposes first, so head 0's SWT
        # (which only needs qT[0]) unblocks after 8 transposes, not 32.
        for half, is_q in ((0, True), (1, True), (0, False), (1, False)):
            for nt in range(8):
                csl = slice(nt * 128, (nt + 1) * 128)
                base = half * 128 if is_q else 256 + half * 128
                pt = pslog.tile([128, 128], F32, tag="log")
                nc.tensor.transpose(
                    pt, rows_all[:, nt, base:base + 128], ident)
                if is_q:
                    nc.vector.tensor_scalar_mul(qT[half][:, csl], pt, SCALE)
                else:
                    nc.vector.tensor_copy(kT[half][:, csl], pt)

        out_sb = persist.tile([128, 8, 256], F32)

        # ---------------- per-head pipeline, 2 groups of 4 heads ---------
        # sw_all holds SWT per head; the shifted-window gather runs as 32
        # DMAs covering 4 heads at once (HWDGE fixed cost is per-DMA).
        # Group 1's SWT matmuls and window DMAs are EMITTED interleaved into
        # group 0's head blocks: engines execute in program order, so this is
        # what lets them overlap group 0's compute.
        sw_all = persist.tile([63, NH, N], F32R)
        qaug_all = persist.tile([64, NH, N], F32R)

        def emit_swt(h):
            qsT = qT[h // 4][(h % 4) * 32:(h % 4) * 32 + 32, :]
            nc.vector.tensor_copy(qaug_all[0:32, h, :], qsT)
            # SWT = rel_w^T @ qs^T + rel_h^T @ qs^T(row-permuted), as ONE
            # K=64 matmul per half: permuted qs staged into qaug rows 32:64
            # (the window gather overwrites those rows afterwards; Tile's WAR
            # tracking orders gather after these matmuls).
            qs0 = qaug_all[0:32, h, :]
            qs0_perm = qs0.rearrange("d (i j) -> d j i", i=32, j=32)
            nc.vector.tensor_copy(
                qaug_all[32:64, h, :].rearrange("d (i j) -> d i j", i=32),
                qs0_perm)
            ps_sw = pssw.tile([63, N], F32, tag="sw", name=f"ps_sw{h}")
            for half in range(2):
                sl = slice(half * 512, (half + 1) * 512)
                nc.tensor.matmul(
                    ps_sw[:, sl], lhsT=rel_st,
                    rhs=qaug_all[0:64, h, sl], start=True, stop=True)
            nc.vector.tensor_copy(sw_all[:, h, :], ps_sw)

        def emit_kaug(h):
            ksT = kT[h // 4][(h % 4) * 32:(h % 4) * 32 + 32, :]
            nc.gpsimd.tensor_copy(
                ka[h % 4][0:32].rearrange("d j m -> d (j m)"), ksT)

        # Shifted-window gather via a DRAM round-trip: in DRAM the
        # partition<->offset coupling of the diagonal becomes plain strides,
        # so ONE DMA per head gathers all 32 windows (vs 32 DMAs each).
        sw_dram = dramp.tile([63, NH, N], F32R)

        def emit_upload(h):
            nc.sync.dma_start(
                out=sw_dram[:, h:h + 1, :], in_=sw_all[:, h:h + 1, :])

        def emit_gather(h):
            # src[t, i, j] = sw_dram[t+31-i, h, i*32+j]
            gsrc = bass.AP(
                tensor=sw_dram.tensor,
                offset=31 * (NH * N) + h * N,
                ap=[[NH * N, 32], [32 - NH * N, 32], [1, 32]])
            nc.sync.dma_start(out=qaug_all[32:64, h, :], in_=gsrc)

        for hh in range(4):
            emit_swt(hh)
            emit_upload(hh)
            emit_gather(hh)
            emit_kaug(hh)

        def flush_outT(pending):
            av2, hpair = pending
            for nt in range(8):
                ps_t = pssw.tile([128, 97], F32, tag="sw")
                nc.tensor.transpose(
                    ps_t, av2[0:97, nt * 128:(nt + 1) * 128],
                    ident[0:97, 0:97])
                for e in range(2):
                    hh = hpair + e
                    rec = stagep.tile([128, 1], F32, tag="rec")
                    nc.vector.reciprocal(
                        rec, ps_t[:, e * 64 + 32:e * 64 + 33])
                    nc.vector.tensor_scalar_mul(
                        out_sb[:, nt, hh * 32:(hh + 1) * 32],
                        ps_t[:, e * 64:e * 64 + 32], rec)
            # ship this pair's 64 output columns while later heads compute
            pair_dst = bass.AP(
                tensor=out.tensor, offset=hpair * 32,
                ap=[[256, 128], [128 * 256, 8], [1, 64]])
            nc.sync.dma_start(
                out=pair_dst, in_=out_sb[:, :, hpair * 32:hpair * 32 + 64])

        pending_outT = None
        for h in range(NH):
            if True:
                if h % 2 == 0:
                    av2_cur = stagep.tile([97, N], F32, tag="av2")
                qaug = qaug_all[:, h, :]
                kaug = ka[h % 4]
                # logits^T m-tiles -> exp -> attn@v accumulation
                ps_a = psatt.tile([33, N], F32, tag="att")
                for j in range(8):
                    ps_l = pslog.tile([128, N], F32, tag="log")
                    for half in range(2):
                        sl = slice(half * 512, (half + 1) * 512)
                        nc.tensor.matmul(
                            ps_l[:, sl], lhsT=kaug[:, j, :],
                            rhs=qaug[:, sl], start=True, stop=True)
                    ew = expwp.tile([128, N], F32R, tag="ew")
                    nc.scalar.activation(ew, ps_l, AF.Exp)
                    for half in range(2):
                        sl = slice(half * 512, (half + 1) * 512)
                        nc.tensor.matmul(
                            ps_a[:, sl], lhsT=v_aug[:, h, j, :],
                            rhs=ew[:, sl],
                            start=(j == 0), stop=(j == 7))
                    if j == 2 and pending_outT is not None:
                        flush_outT(pending_outT)
                        pending_outT = None
                    if j == 4 and h + 4 < NH:
                        emit_swt(h + 4)
                        emit_upload(h + 4)
                        emit_gather(h + 4)

                if h + 4 < NH:
                    emit_kaug(h + 4)

                # stage attn output; transpose+normalize per PAIR of heads
                av2 = av2_cur
                nc.vector.tensor_copy(
                    av2[(h % 2) * 64:(h % 2) * 64 + 33, :], ps_a)
                if h % 2 == 1:
                    pending_outT = (av2, h - 1)


        if pending_outT is not None:
            flush_outT(pending_outT)
            pending_outT = None


_NC_CACHE = {}


def _build():
    if "nc" in _NC_CACHE:
        return _NC_CACHE["nc"]
    nc = bass.Bass("TRN2", target_bir_lowering=False, debug=False,
                   enable_asserts=True, num_devices=8)
    ins = {
        "x": nc.dram_tensor("x", [N, 768], F32, kind="ExternalInput").ap(),
        "relh": nc.dram_tensor("relh", [63, 32], F32, kind="ExternalInput").ap(),
        "relw": nc.dram_tensor("relw", [63, 32], F32, kind="ExternalInput").ap(),
    }
    outs = {
        "out": nc.dram_tensor("out", [N, 256], F32, kind="ExternalOutput").ap(),
    }
    with tile.TileContext(nc) as tc:
        kernel_body(tc, outs, ins)
    split_multiwaits(nc)
    _NC_CACHE["nc"] = nc
    return nc


def kernel(inputs, key_rel_h, key_rel_w, _trace=False):
    nc = _build()
    x = np.ascontiguousarray(np.asarray(inputs, dtype=np.float32).reshape(8, N, 768))
    rh = np.ascontiguousarray(np.asarray(key_rel_h, dtype=np.float32))
    rw = np.ascontiguousarray(np.asarray(key_rel_w, dtype=np.float32))
    in_maps = [{"x": x[c], "relh": rh, "relw": rw} for c in range(8)]
    res = bass_utils.run_bass_kernel_spmd(
        nc, in_maps, core_ids=list(range(8)), trace=_trace)
    outp = np.stack([r["out"] for r in res.results])
    if _trace:
        kernel.last_results = res
    return outp.reshape(8, 32, 32, 256)



# revision 32
# speedup vs baseline: 1.1724x; 1.1724x over previous
"""AttentionAugmentation2D Trainium2 kernel (v5).

Shapes (hardcoded): B=8, H=W=32, N=1024, NH=8 heads, dk=dv=32 per head.
inputs [8,32,32,768] = q|k|v (256 each), key_rel_h/w [63,32].

Sharding: data-parallel over batch B across the 8 cores. Each core runs the
full 8-head attention for its batch.

Math per (batch, head), with n=(i,j), m=(i',j') (i = H index):
  logits[n,m] = q[n]@k[m] + q[(j,i)]@rel_h[i'-i+31] + q[(i,j)]@rel_w[i'-i+31]
Both rel terms depend on m only through i', so with
  SWT[u,n] = rel_w[u]@q[(i,j)] + rel_h[u]@q[(j,i)]        (u in [0,63))
  biasT[t,n] = SWT[t+31-i(n), n]                          (shifted windows)
we get  logits^T = K_aug^T.T @ Q_augT  with contraction 64:
  K_aug^T rows: 0:32 = k^T, 32:64 = onehot[t==i'(m)]
  Q_augT rows:  0:32 = q^T, 32:64 = biasT
biasT is computed directly as 32 small matmuls (one per i-block of n),
each using a shifted 32-column window of rel^T as the stationary operand —
no DRAM round-trip. These run in bf16 (a 32-wide free dim would be 4x
slower in f32r on the PE); the bias is small relative to q@k so bf16 is
plenty.  The 1/sqrt(dk) q-scale is folded into the exp activation's scale
operand (logits are linear in q). Softmax without max-subtraction (scaled
logits bounded ~+-8 for randn inputs); row sums come free from a
ones-column appended to V in the attn@V matmul.

Pipeline design (cost-model driven): the Act engine's 64 exps (~66us) are
the floor; everything else is arranged to hide behind them:
 - head-0 critical chain front-loaded and engine-balanced: q-half0 DMA ->
   PE transposes -> {bf16 q^T copy (DVE) || permuted copy (Pool)} -> bias
   matmuls (PE) -> PSUM copies (DVE) -> first logits; dummy PE matmuls at
   t=0 keep the PE p-state ramp warm before the first real transpose;
 - heads 0/1 run their bias matmuls inside the then-unused odd attn@V
   PSUM slot, so the two shared scratch PSUM banks never sit between the
   q/k transposes and the first logits;
 - rel/v/half-1-row DMAs ride the Act-engine DMA queue so the SP queue
   serves nothing but the critical row loads (and later output stores);
 - half-1 input transposes emitted inside head 0's j-loop;
 - attn@V accumulates into a manually double-buffered PSUM region
   (partition slots 0:33 / 64:97);
 - per-head attn^T copy to SBUF at head end; transpose+normalize flush of
   head h interleaved into head h+1's j-loop; outputs shipped as quad
   DMAs (pair DMAs on the final head to shorten the tail).

Toolchain note: walrus codegen only fits ONE semaphore wait in most TPB
instruction structs and does not split excess waits itself (stock kernels
trip this too).  split_multiwaits() below is a BIR post-pass that moves
excess waits onto same-engine InstNoOp carriers placed immediately before
the offending instruction — semantically identical, compiles everywhere.
"""

import numpy as np

import concourse.bass as bass
import concourse.mybir as mybir
import concourse.tile as tile
from concourse import bass_utils
from concourse.masks import make_identity
from concourse.tile import add_dep_helper

F32 = mybir.dt.float32
F32R = mybir.dt.float32r
BF16 = mybir.dt.bfloat16
AF = mybir.ActivationFunctionType

NH = 8
N = 1024
DK = 32
SCALE = float(DK) ** -0.5


def split_multiwaits(nc, dma_limit=1):
    """Move excess semaphore waits onto same-engine nop carriers."""
    n_new = 0
    for f in nc.m.functions:
        for blk in f.blocks:
            newlist = []
            for inst in blk.instructions:
                si = getattr(inst, "sync_info", None)
                is_dma = isinstance(inst, mybir.InstDMACopy)
                limit = dma_limit if is_dma else 1
                if si is not None and len(si.on_wait) > limit:
                    waits = list(si.on_wait)
                    for w in waits[:-1]:
                        n_new += 1
                        newlist.append(mybir.InstNoOp(
                            name=f"I-wc{n_new}",
                            ins=[], outs=[],
                            sync_info=mybir.SyncInfo(on_wait=[w], on_update=[]),
                            bass_nofuse=True,
                            engine=inst.engine,
                        ))
                    inst.sync_info = mybir.SyncInfo(
                        on_wait=waits[-1:], on_update=si.on_update)
                newlist.append(inst)
            blk.instructions = newlist
    return n_new


def kernel_body(tc, outs, ins):
    nc = tc.nc
    x = ins["x"]          # [1024, 768] rows n=(i,j), cols q|k|v
    relh = ins["relh"]    # [63, 32]
    relw = ins["relw"]    # [63, 32]
    out = outs["out"]     # [1024, 256]

    with (
        tc.tile_pool(name="persist", bufs=1) as persist,
        tc.tile_pool(name="expw", bufs=4) as expwp,
        tc.tile_pool(name="qbfp", bufs=2) as qbfp,
        tc.tile_pool(name="av2p", bufs=2) as av2p,
        tc.tile_pool(name="stage", bufs=4) as stagep,
        tc.tile_pool(name="psum_log", bufs=2, space="PSUM") as pslog,
        tc.tile_pool(name="psum_scr", bufs=2, space="PSUM") as psscr,
        tc.tile_pool(name="psum_att", bufs=1, space="PSUM") as psattp,
    ):
        # attn@V accumulator: one 2-bank region, manually double-buffered by
        # partition slot (head h -> rows (h%2)*64 + 0:33). The odd slot also
        # hosts the p-state warm-up dummies and heads 0/1's bias matmuls
        # (its first real use, head 1's attn@V, comes ~20us in).
        ps_att = psattp.tile([128, N], F32, tag="att")

        # ---------------- warm-up + constants ----------------
        dummy_sb = persist.tile([128, 64], F32)
        nc.vector.memset(dummy_sb, 0.0)
        for w in range(22):
            nc.tensor.matmul(ps_att[64:96, 0:64], lhsT=dummy_sb[:, 0:32],
                             rhs=dummy_sb, start=True, stop=True)

        ident = persist.tile([128, 128], F32)
        make_identity(nc, ident)
        # f32r operands must come from f32r-rounding writes (BIR verifier);
        # a DVE copy provides the rounded replica used by all transposes
        identR = persist.tile([128, 128], F32R)
        nc.vector.tensor_copy(identR, ident)
        ident_marker = nc.gpsimd.tensor_copy(ident[0:1, 0:1], ident[0:1, 0:1])

        # rel tables + all non-critical loads on the Act DMA queue; SP
        # carries only the head-0-critical row loads and output stores.
        rel_st = persist.tile([64, 63], F32R)
        nc.scalar.dma_start(out=rel_st[0:32],
                            in_=relw.rearrange("u d -> d u").bitcast(F32R))
        nc.scalar.dma_start(out=rel_st[32:64],
                            in_=relh.rearrange("u d -> d u").bitcast(F32R))

        rowsR = persist.tile([128, 4, 8, 128], F32R)
        CB_COLS = (0, 256, 128, 384)   # q0, k0, q1, k1

        def emit_rows_dma(cb, eng, lo=0, hi=8):
            src = bass.AP(tensor=x.tensor,
                          offset=CB_COLS[cb] + lo * 128 * 768,
                          ap=[[768, 128], [128 * 768, hi - lo], [1, 128]],
                          ).bitcast(F32R)
            eng.dma_start(out=rowsR[:, cb, lo:hi, :], in_=src)

        emit_rows_dma(0, nc.sync, 0, 4)
        emit_rows_dma(0, nc.sync, 4, 8)
        emit_rows_dma(1, nc.sync)
        emit_rows_dma(2, nc.scalar)    # q-half1
        emit_rows_dma(3, nc.scalar)    # k-half1

        # The Tile scheduler orders each engine's queue with its own internal
        # timing model, which disagrees with the device timing enough to
        # shuffle the Pool queue badly (observed: kaug0 scheduled after five
        # unrelated 1.5us Pool copies, gating the first logits by ~8us).
        # Chain every Pool op (and the startup DVE ops) in emission order
        # with same-engine ordering-only deps.
        _chain = {"pool": ident_marker}

        def chained(engine_name, inst):
            prev = _chain.get(engine_name)
            if prev is not None:
                add_dep_helper(inst.ins, prev.ins, sync=False,
                               reason=f"pin {engine_name} queue order")
            _chain[engine_name] = inst
            return inst

        def pool(op, *args, **kwargs):
            return chained("pool", getattr(nc.gpsimd, op)(*args, **kwargs))

        # onehot for K_aug rows 32:64 (Pool; ka[0]'s copy early for head 0,
        # the rest fed into the chain later, outside the critical window)
        oh_st = persist.tile([32, 8, 128], F32)
        pool("memset", oh_st, 0.0)
        oh = oh_st.rearrange("t j (b m) -> t j b m", b=4)
        pool("affine_select",
             out=oh, in_=oh, compare_op=mybir.AluOpType.not_equal,
             fill=1.0, base=0, pattern=[[-4, 8], [-1, 4], [0, 32]],
             channel_multiplier=1)
        ones_st = persist.tile([128, 64], F32)
        pool("memset", ones_st, 1.0)

        rel_bf = persist.tile([64, 63], BF16)

        ka = [persist.tile([64, 8, 128], F32R, tag=f"ka{i}", name=f"ka{i}")
              for i in range(4)]
        pool("tensor_copy", ka[0][32:64], oh_st)
        pool("tensor_copy", rel_bf, rel_st)

        # qT/kT: [p, half, n]; partitions = 32*(h%4)+d within a half
        qT = persist.tile([128, 2, N], F32R)
        kT = persist.tile([128, 2, N], F32R)

        def emit_transposes(cb, dst, half, lo=None, pin=False,
                            pe_anchor=None):
            # 4 transposes into one scratch bank + a single wide copy: the
            # copy's fixed PSUM-access cost is paid once per 4 tiles
            groups = (0, 4) if lo is None else (lo,)
            for g in groups:
                pt = psscr.tile([128, 512], F32R, tag="scr", name=f"pt{cb}_{g}")
                for c in range(4):
                    tr = nc.tensor.transpose(pt[:, c * 128:(c + 1) * 128],
                                             rowsR[:, cb, g + c, :], identR)
                    if pe_anchor is not None:
                        add_dep_helper(tr.ins, pe_anchor.ins, sync=False,
                                       reason="filler after this j's attn@V")
                cp = nc.vector.tensor_copy(
                    dst[:, half, g * 128:(g + 4) * 128], pt)
                if pin:
                    chained("dve", cp)

        # ---------------- per-head q staging + shifted-window bias -------
        qaug_all = persist.tile([64, NH, N], F32R)

        def emit_bias_stage_a(h, pin_dve=False):
            """bf16 staging [q^T ; q^T row-permuted] + f32r q^T for logits."""
            lane = (h % 4) * 32
            qsT = qT[lane:lane + 32, h // 4, :]
            qbf = qbfp.tile([64, N], BF16, tag="qbf", name=f"qbf{h}")
            cv = nc.vector.tensor_copy(qbf[0:32, :], qsT)
            if pin_dve:
                chained("dve", cv)
            pool("tensor_copy",
                 qbf[32:64, :].rearrange("d (i j) -> d i j", i=32),
                 qsT.rearrange("d (i j) -> d j i", i=32, j=32))
            pool("tensor_copy", qaug_all[0:32, h, :], qsT)
            return qbf

        def emit_bias_stage_b(h, qbf, ps_bs, pin_dve=False,
                              pe_anchor=None):
            """biasT[t, (i,j)] = SWT[t+31-i, (i,j)]: per i-block matmul with
            a shifted rel^T window as the stationary operand."""
            for half in range(2):
                ps_b = ps_bs[half]
                for ib in range(16):
                    i = half * 16 + ib
                    mm = nc.tensor.matmul(
                        ps_b[:, ib * 32:(ib + 1) * 32],
                        lhsT=rel_bf[:, 31 - i:63 - i],
                        rhs=qbf[:, i * 32:(i + 1) * 32],
                        start=True, stop=True)
                    if pe_anchor is not None:
                        add_dep_helper(mm.ins, pe_anchor.ins, sync=False,
                                       reason="filler after this j's attn@V")
                cp = nc.vector.tensor_copy(
                    qaug_all[32:64, h, half * 512:(half + 1) * 512], ps_b)
                if pin_dve:
                    chained("dve", cp)

        def emit_bias(h, pin_dve=False, pe_anchor=None):
            qbf = emit_bias_stage_a(h, pin_dve)
            ps_bs = [psscr.tile([32, 512], F32, tag="scr", name=f"ps_b{h}_{x}")
                     for x in range(2)]
            emit_bias_stage_b(h, qbf, ps_bs, pin_dve, pe_anchor)

        def emit_kaug(h, split=False):
            lane = (h % 4) * 32
            ksT = kT[lane:lane + 32, h // 4, :]
            if split:
                # j0/j1 slice first so the first logits unblock early
                pool("tensor_copy",
                     ka[h % 4][0:32, 0:2, :].rearrange("d j m -> d (j m)"),
                     ksT[:, 0:256])
                pool("tensor_copy",
                     ka[h % 4][0:32, 2:8, :].rearrange("d j m -> d (j m)"),
                     ksT[:, 256:1024])
            else:
                pool("tensor_copy",
                     ka[h % 4][0:32].rearrange("d j m -> d (j m)"), ksT)

        def emit_onehot(i):
            nc.vector.tensor_copy(ka[i][32:64], oh_st)

        # ---------------- startup emission order ----------------
        # Strictly head-0-critical work before the loop; head 0's bias runs
        # in the then-unused odd attn@V PSUM slot so the two scratch banks
        # stay free for the q0/k0 transposes. Everything else (bias/kaug/
        # onehot for later heads) is fed into the j-loop hooks.
        emit_transposes(0, qT, 0, pin=True)      # q-half0
        qbf0 = emit_bias_stage_a(0, pin_dve=True)
        emit_transposes(1, kT, 0, pin=True)      # k-half0
        emit_bias_stage_b(0, qbf0, [ps_att[64:96, 0:512],
                                    ps_att[64:96, 512:1024]], pin_dve=True)
        emit_kaug(0, split=True)

        # v chunks staged in f32 (contiguous DMAs on the Act queue) and
        # converted to bf16 on Pool: the attn@V matmul accumulates into a
        # partition-offset PSUM slot, which the ISA allows for bf16 but not
        # f32r operands. bf16 exp-weights/v cost ~1e-3 relative error.
        v_st = persist.tile([128, 8, 256], F32)
        v_aug = persist.tile([128, NH, 8, 33], BF16)
        for j in range(8):
            nc.scalar.dma_start(
                out=v_st[:, j, :], in_=x[j * 128:(j + 1) * 128, 512:768])

        def emit_vconv(j):
            nc.vector.tensor_copy(
                v_aug[:, :, j, 0:32],
                v_st[:, j, :].rearrange("p (h d) -> p h d", h=NH))

        emit_vconv(0)
        nc.vector.tensor_copy(
            v_aug[:, :, :, 32:33].rearrange("p h j o -> p (h j o)"), ones_st)
        emit_vconv(1)
        emit_vconv(2)

        # ---------------- output staging / flush ----------------
        out_sb = persist.tile([128, 8, 256], F32)

        def emit_av2_copy(h, av2t, csl=slice(0, N)):
            s = (h % 2) * 64
            nc.vector.tensor_copy(av2t[0:33, csl], ps_att[s:s + 33, csl])

        def flush_head(h, av2t, nts, tail=False):
            groups = {3: 4, 7: 4}
            for nt in nts:
                csl = slice(nt * 128, (nt + 1) * 128)
                # on the tail, borrow the (by then idle) logits PSUM banks so
                # four transposes can be in flight instead of two
                pool_ = pslog if (tail and nt % 2) else psscr
                tag = "log" if (tail and nt % 2) else "scr"
                ps_t = pool_.tile([128, 33], F32, tag=tag, name=f"ps_t{h}_{nt}")
                nc.tensor.transpose(ps_t, av2t[0:33, csl], ident[0:33, 0:33])
                rec = stagep.tile([128, 1], F32, tag="rec")
                nc.vector.reciprocal(rec, ps_t[:, 32:33])
                nc.vector.tensor_scalar_mul(
                    out_sb[:, nt, h * 32:(h + 1) * 32], ps_t[:, 0:32], rec)
                if nt in groups:
                    group = groups[nt]
                    g = nt - group + 1
                    dstap = bass.AP(
                        tensor=out.tensor,
                        offset=g * 128 * 256 + h * 32,
                        ap=[[256, 128], [128 * 256, group], [1, 32]])
                    nc.sync.dma_start(
                        out=dstap, in_=out_sb[:, g:nt + 1, h * 32:(h + 1) * 32])

        # ---------------- per-head pipeline ----------------
        # early-head hooks: later heads' staging spread across the j-loops
        # so nothing competes with the first heads' critical chains
        HOOKS = {
            (0, 2): lambda a: (emit_vconv(3), emit_vconv(4),
                               emit_transposes(2, qT, 1, 0, pin=True,
                                               pe_anchor=a)),
            (0, 3): lambda a: (emit_bias(1, pin_dve=True, pe_anchor=a),
                               emit_transposes(2, qT, 1, 4, pin=True,
                                               pe_anchor=a)),
            (0, 4): lambda a: (emit_vconv(5), emit_vconv(6),
                               emit_vconv(7), emit_kaug(1), emit_onehot(1)),
            (0, 5): lambda a: (emit_bias(2, pe_anchor=a),
                               emit_transposes(3, kT, 1, 0, pin=True,
                                               pe_anchor=a)),
            (0, 6): lambda a: emit_transposes(3, kT, 1, 4, pin=True,
                                              pe_anchor=a),
            (1, 1): lambda a: (emit_kaug(2), emit_onehot(2)),
            (1, 3): lambda a: (emit_kaug(3), emit_onehot(3), emit_kaug(4)),
            (2, 1): lambda a: emit_kaug(5),
            (3, 1): lambda a: emit_kaug(6),
            (4, 1): lambda a: emit_kaug(7),
        }
        pending = None    # (head, av2t) awaiting transpose+normalize
        for h in range(NH):
            s = (h % 2) * 64
            qaug = qaug_all[:, h, :]
            kaug = ka[h % 4]
            for j in range(8):
                ps_l = pslog.tile([128, N], F32, tag="log", name=f"ps_l{h}_{j}")
                for half in range(2):
                    sl = slice(half * 512, (half + 1) * 512)
                    nc.tensor.matmul(
                        ps_l[:, sl], lhsT=kaug[:, j, :],
                        rhs=qaug[:, sl], start=True, stop=True)
                ew = expwp.tile([128, N], BF16, tag="ew", name=f"ew{h}_{j}")
                nc.scalar.activation(ew, ps_l, AF.Exp, scale=SCALE)
                for half in range(2):
                    sl = slice(half * 512, (half + 1) * 512)
                    a_last = nc.tensor.matmul(
                        ps_att[s:s + 33, sl], lhsT=v_aug[:, h, j, :],
                        rhs=ew[:, sl], start=(j == 0), stop=(j == 7))
                if (h, j) in HOOKS:
                    HOOKS[(h, j)](a_last)
                if j == 2 and pending is not None:
                    flush_head(pending[0], pending[1], range(8))
                    pending = None
                if j == 5 and h >= 1 and h + 2 < NH:
                    emit_bias(h + 2)

            av2t = av2p.tile([33, N], F32, tag="av2", name=f"av2_{h}")
            if h < NH - 1:
                emit_av2_copy(h, av2t)
            else:
                # tail: quarter-split the last copy so the flush transposes
                # start as soon as the first columns land
                for qq in range(4):
                    emit_av2_copy(h, av2t, slice(qq * 256, (qq + 1) * 256))
            pending = (h, av2t)

        # tail flush of the last head
        flush_head(pending[0], pending[1], range(4), tail=True)
        flush_head(pending[0], pending[1], range(4, 8), tail=True)


_NC_CACHE = {}


def _build():
    if "nc" in _NC_CACHE:
        return _NC_CACHE["nc"]
    nc = bass.Bass("TRN2", target_bir_lowering=False, debug=False,
                   enable_asserts=True, num_devices=8)
    ins = {
        "x": nc.dram_tensor("x", [N, 768], F32, kind="ExternalInput").ap(),
        "relh": nc.dram_tensor("relh", [63, 32], F32, kind="ExternalInput").ap(),
        "relw": nc.dram_tensor("relw", [63, 32], F32, kind="ExternalInput").ap(),
    }
    outs = {
        "out": nc.dram_tensor("out", [N, 256], F32, kind="ExternalOutput").ap(),
    }
    with tile.TileContext(nc) as tc:
        kernel_body(tc, outs, ins)
    split_multiwaits(nc)
    _NC_CACHE["nc"] = nc
    return nc


def kernel(inputs, key_rel_h, key_rel_w, _trace=False):
    nc = _build()
    x = np.ascontiguousarray(np.asarray(inputs, dtype=np.float32).reshape(8, N, 768))
    rh = np.ascontiguousarray(np.asarray(key_rel_h, dtype=np.float32))
    rw = np.ascontiguousarray(np.asarray(key_rel_w, dtype=np.float32))
    in_maps = [{"x": x[c], "relh": rh, "relw": rw} for c in range(8)]
    res = bass_utils.run_bass_kernel_spmd(
        nc, in_maps, core_ids=list(range(8)), trace=_trace)
    outp = np.stack([r["out"] for r in res.results])
    if _trace:
        kernel.last_results = res
    return outp.reshape(8, 32, 32, 256)


# revision 38
# speedup vs baseline: 1.1806x; 1.0070x over previous
"""AttentionAugmentation2D Trainium2 kernel (v5).

Shapes (hardcoded): B=8, H=W=32, N=1024, NH=8 heads, dk=dv=32 per head.
inputs [8,32,32,768] = q|k|v (256 each), key_rel_h/w [63,32].

Sharding: data-parallel over batch B across the 8 cores. Each core runs the
full 8-head attention for its batch.

Math per (batch, head), with n=(i,j), m=(i',j') (i = H index):
  logits[n,m] = q[n]@k[m] + q[(j,i)]@rel_h[i'-i+31] + q[(i,j)]@rel_w[i'-i+31]
Both rel terms depend on m only through i', so with
  SWT[u,n] = rel_w[u]@q[(i,j)] + rel_h[u]@q[(j,i)]        (u in [0,63))
  biasT[t,n] = SWT[t+31-i(n), n]                          (shifted windows)
we get  logits^T = K_aug^T.T @ Q_augT  with contraction 64:
  K_aug^T rows: 0:32 = k^T, 32:64 = onehot[t==i'(m)]
  Q_augT rows:  0:32 = q^T, 32:64 = biasT
biasT is computed directly as 32 small matmuls (one per i-block of n),
each using a shifted 32-column window of rel^T as the stationary operand —
no DRAM round-trip. These run in bf16 (a 32-wide free dim would be 4x
slower in f32r on the PE); the bias is small relative to q@k so bf16 is
plenty.  The 1/sqrt(dk) q-scale is folded into the exp activation's scale
operand (logits are linear in q). Softmax without max-subtraction (scaled
logits bounded ~+-8 for randn inputs); row sums come free from a
ones-column appended to V in the attn@V matmul.

Pipeline design (cost-model driven): the Act engine's 64 exps (~66us) are
the floor; everything else is arranged to hide behind them:
 - head-0 critical chain front-loaded and engine-balanced: q-half0 DMA ->
   PE transposes -> {bf16 q^T copy (DVE) || permuted copy (Pool)} -> bias
   matmuls (PE) -> PSUM copies (DVE) -> first logits; dummy PE matmuls at
   t=0 keep the PE p-state ramp warm before the first real transpose;
 - heads 0/1 run their bias matmuls inside the then-unused odd attn@V
   PSUM slot, so the two shared scratch PSUM banks never sit between the
   q/k transposes and the first logits;
 - rel/v/half-1-row DMAs ride the Act-engine DMA queue so the SP queue
   serves nothing but the critical row loads (and later output stores);
 - half-1 input transposes emitted inside head 0's j-loop;
 - attn@V accumulates into a manually double-buffered PSUM region
   (partition slots 0:33 / 64:97);
 - per-head attn^T copy to SBUF at head end; transpose+normalize flush of
   head h interleaved into head h+1's j-loop; outputs shipped as quad
   DMAs (pair DMAs on the final head to shorten the tail).

Toolchain note: walrus codegen only fits ONE semaphore wait in most TPB
instruction structs and does not split excess waits itself (stock kernels
trip this too).  split_multiwaits() below is a BIR post-pass that moves
excess waits onto same-engine InstNoOp carriers placed immediately before
the offending instruction — semantically identical, compiles everywhere.
"""

import numpy as np

import concourse.bass as bass
import concourse.mybir as mybir
import concourse.tile as tile
from concourse import bass_utils
from concourse.masks import make_identity
from concourse.tile import add_dep_helper

F32 = mybir.dt.float32
F32R = mybir.dt.float32r
BF16 = mybir.dt.bfloat16
AF = mybir.ActivationFunctionType

NH = 8
N = 1024
DK = 32
SCALE = float(DK) ** -0.5


def split_multiwaits(nc, dma_limit=1):
    """Move excess semaphore waits onto same-engine nop carriers."""
    n_new = 0
    for f in nc.m.functions:
        for blk in f.blocks:
            newlist = []
            for inst in blk.instructions:
                si = getattr(inst, "sync_info", None)
                is_dma = isinstance(inst, mybir.InstDMACopy)
                limit = dma_limit if is_dma else 1
                if si is not None and len(si.on_wait) > limit:
                    waits = list(si.on_wait)
                    for w in waits[:-1]:
                        n_new += 1
                        newlist.append(mybir.InstNoOp(
                            name=f"I-wc{n_new}",
                            ins=[], outs=[],
                            sync_info=mybir.SyncInfo(on_wait=[w], on_update=[]),
                            bass_nofuse=True,
                            engine=inst.engine,
                        ))
                    inst.sync_info = mybir.SyncInfo(
                        on_wait=waits[-1:], on_update=si.on_update)
                newlist.append(inst)
            blk.instructions = newlist
    return n_new


def kernel_body(tc, outs, ins):
    nc = tc.nc
    x = ins["x"]          # [1024, 768] rows n=(i,j), cols q|k|v
    relh = ins["relh"]    # [63, 32]
    relw = ins["relw"]    # [63, 32]
    out = outs["out"]     # [1024, 256]

    with (
        tc.tile_pool(name="persist", bufs=1) as persist,
        tc.tile_pool(name="expw", bufs=4) as expwp,
        tc.tile_pool(name="qbfp", bufs=2) as qbfp,
        tc.tile_pool(name="av2p", bufs=2) as av2p,
        tc.tile_pool(name="stage", bufs=4) as stagep,
        tc.tile_pool(name="psum_log", bufs=2, space="PSUM") as pslog,
        tc.tile_pool(name="psum_scr", bufs=2, space="PSUM") as psscr,
        tc.tile_pool(name="psum_att", bufs=1, space="PSUM") as psattp,
    ):
        # attn@V accumulator: one 2-bank region, manually double-buffered by
        # partition slot (head h -> rows (h%2)*64 + 0:33). The odd slot also
        # hosts the p-state warm-up dummies and heads 0/1's bias matmuls
        # (its first real use, head 1's attn@V, comes ~20us in).
        ps_att = psattp.tile([128, N], F32, tag="att")

        # ---------------- warm-up + constants ----------------
        dummy_sb = persist.tile([128, 64], F32)
        nc.vector.memset(dummy_sb, 0.0)
        for w in range(22):
            nc.tensor.matmul(ps_att[64:96, 0:64], lhsT=dummy_sb[:, 0:32],
                             rhs=dummy_sb, start=True, stop=True)

        ident = persist.tile([128, 128], F32)
        make_identity(nc, ident)
        # f32r operands must come from f32r-rounding writes (BIR verifier);
        # a DVE copy provides the rounded replica used by all transposes
        identR = persist.tile([128, 128], F32R)
        nc.vector.tensor_copy(identR, ident)
        ident_marker = nc.gpsimd.tensor_copy(ident[0:1, 0:1], ident[0:1, 0:1])

        # rel tables + all non-critical loads on the Act DMA queue; SP
        # carries only the head-0-critical row loads and output stores.
        rel_st = persist.tile([64, 63], F32R)
        nc.scalar.dma_start(out=rel_st[0:32],
                            in_=relw.rearrange("u d -> d u").bitcast(F32R))
        nc.scalar.dma_start(out=rel_st[32:64],
                            in_=relh.rearrange("u d -> d u").bitcast(F32R))

        rowsR = persist.tile([128, 4, 8, 128], F32R)
        CB_COLS = (0, 256, 128, 384)   # q0, k0, q1, k1

        def emit_rows_dma(cb, eng, lo=0, hi=8):
            src = bass.AP(tensor=x.tensor,
                          offset=CB_COLS[cb] + lo * 128 * 768,
                          ap=[[768, 128], [128 * 768, hi - lo], [1, 128]],
                          ).bitcast(F32R)
            eng.dma_start(out=rowsR[:, cb, lo:hi, :], in_=src)

        emit_rows_dma(0, nc.sync, 0, 4)
        emit_rows_dma(0, nc.sync, 4, 8)
        emit_rows_dma(1, nc.sync)
        emit_rows_dma(2, nc.scalar)    # q-half1
        emit_rows_dma(3, nc.scalar)    # k-half1

        # The Tile scheduler orders each engine's queue with its own internal
        # timing model, which disagrees with the device timing enough to
        # shuffle the Pool queue badly (observed: kaug0 scheduled after five
        # unrelated 1.5us Pool copies, gating the first logits by ~8us).
        # Chain every Pool op (and the startup DVE ops) in emission order
        # with same-engine ordering-only deps.
        _chain = {"pool": ident_marker}

        def chained(engine_name, inst):
            prev = _chain.get(engine_name)
            if prev is not None:
                add_dep_helper(inst.ins, prev.ins, sync=False,
                               reason=f"pin {engine_name} queue order")
            _chain[engine_name] = inst
            return inst

        def pool(op, *args, **kwargs):
            return chained("pool", getattr(nc.gpsimd, op)(*args, **kwargs))

        # onehot for K_aug rows 32:64 (Pool; ka[0]'s copy early for head 0,
        # the rest fed into the chain later, outside the critical window)
        oh_st = persist.tile([32, 8, 128], F32)
        pool("memset", oh_st, 0.0)
        oh = oh_st.rearrange("t j (b m) -> t j b m", b=4)
        pool("affine_select",
             out=oh, in_=oh, compare_op=mybir.AluOpType.not_equal,
             fill=1.0, base=0, pattern=[[-4, 8], [-1, 4], [0, 32]],
             channel_multiplier=1)
        ones_st = persist.tile([128, 64], F32)
        pool("memset", ones_st, 1.0)

        rel_bf = persist.tile([64, 63], BF16)

        ka = [persist.tile([64, 8, 128], F32R, tag=f"ka{i}", name=f"ka{i}")
              for i in range(4)]
        pool("tensor_copy", ka[0][32:64], oh_st)
        pool("tensor_copy", rel_bf, rel_st)

        # qT/kT: [p, half, n]; partitions = 32*(h%4)+d within a half
        qT = persist.tile([128, 2, N], F32R)
        kT = persist.tile([128, 2, N], F32R)

        def emit_transposes(cb, dst, half, lo=None, pin=False,
                            pe_anchor=None):
            # 4 transposes into one scratch bank + a single wide copy: the
            # copy's fixed PSUM-access cost is paid once per 4 tiles
            groups = (0, 4) if lo is None else (lo,)
            for g in groups:
                pt = psscr.tile([128, 512], F32R, tag="scr", name=f"pt{cb}_{g}")
                for c in range(4):
                    tr = nc.tensor.transpose(pt[:, c * 128:(c + 1) * 128],
                                             rowsR[:, cb, g + c, :], identR)
                    if pe_anchor is not None:
                        add_dep_helper(tr.ins, pe_anchor.ins, sync=False,
                                       reason="filler after this j's attn@V")
                cp = nc.vector.tensor_copy(
                    dst[:, half, g * 128:(g + 4) * 128], pt)
                if pin:
                    chained("dve", cp)

        # ---------------- per-head q staging + shifted-window bias -------
        qaug_all = persist.tile([64, NH, N], F32R)

        def emit_bias_stage_a(h, pin_dve=False):
            """bf16 staging [q^T ; q^T row-permuted] + f32r q^T for logits."""
            lane = (h % 4) * 32
            qsT = qT[lane:lane + 32, h // 4, :]
            qbf = qbfp.tile([64, N], BF16, tag="qbf", name=f"qbf{h}")
            cv = nc.vector.tensor_copy(qbf[0:32, :], qsT)
            if pin_dve:
                chained("dve", cv)
            pool("tensor_copy",
                 qbf[32:64, :].rearrange("d (i j) -> d i j", i=32),
                 qsT.rearrange("d (i j) -> d j i", i=32, j=32))
            pool("tensor_copy", qaug_all[0:32, h, :], qsT)
            return qbf

        def emit_bias_stage_b(h, qbf, ps_bs, pin_dve=False,
                              pe_anchor=None):
            """biasT[t, (i,j)] = SWT[t+31-i, (i,j)]: per i-block matmul with
            a shifted rel^T window as the stationary operand."""
            for half in range(2):
                ps_b = ps_bs[half]
                for ib in range(16):
                    i = half * 16 + ib
                    mm = nc.tensor.matmul(
                        ps_b[:, ib * 32:(ib + 1) * 32],
                        lhsT=rel_bf[:, 31 - i:63 - i],
                        rhs=qbf[:, i * 32:(i + 1) * 32],
                        start=True, stop=True)
                    if pe_anchor is not None:
                        add_dep_helper(mm.ins, pe_anchor.ins, sync=False,
                                       reason="filler after this j's attn@V")
                cp = nc.vector.tensor_copy(
                    qaug_all[32:64, h, half * 512:(half + 1) * 512], ps_b)
                if pin_dve:
                    chained("dve", cp)

        def emit_bias(h, pin_dve=False, pe_anchor=None):
            qbf = emit_bias_stage_a(h, pin_dve)
            ps_bs = [psscr.tile([32, 512], F32, tag="scr", name=f"ps_b{h}_{x}")
                     for x in range(2)]
            emit_bias_stage_b(h, qbf, ps_bs, pin_dve, pe_anchor)

        def emit_kaug(h, split=False):
            lane = (h % 4) * 32
            ksT = kT[lane:lane + 32, h // 4, :]
            if split:
                # j0/j1 slice on the (faster) DVE chain so the first logits
                # unblock early; the rest follows on Pool
                chained("dve", nc.vector.tensor_copy(
                    ka[h % 4][0:32, 0:2, :].rearrange("d j m -> d (j m)"),
                    ksT[:, 0:256]))
                pool("tensor_copy",
                     ka[h % 4][0:32, 2:8, :].rearrange("d j m -> d (j m)"),
                     ksT[:, 256:1024])
            else:
                pool("tensor_copy",
                     ka[h % 4][0:32].rearrange("d j m -> d (j m)"), ksT)

        def emit_onehot(i):
            nc.vector.tensor_copy(ka[i][32:64], oh_st)

        # ---------------- startup emission order ----------------
        # Strictly head-0-critical work before the loop; head 0's bias runs
        # in the then-unused odd attn@V PSUM slot so the two scratch banks
        # stay free for the q0/k0 transposes. Everything else (bias/kaug/
        # onehot for later heads) is fed into the j-loop hooks.
        emit_transposes(0, qT, 0, pin=True)      # q-half0
        qbf0 = emit_bias_stage_a(0, pin_dve=True)
        emit_transposes(1, kT, 0, pin=True)      # k-half0
        emit_kaug(0, split=True)
        emit_bias_stage_b(0, qbf0, [ps_att[64:96, 0:512],
                                    ps_att[64:96, 512:1024]], pin_dve=True)

        # v chunks staged in f32 (contiguous DMAs on the Act queue) and
        # converted to bf16 on Pool: the attn@V matmul accumulates into a
        # partition-offset PSUM slot, which the ISA allows for bf16 but not
        # f32r operands. bf16 exp-weights/v cost ~1e-3 relative error.
        v_st = persist.tile([128, 8, 256], F32)
        v_aug = persist.tile([128, NH, 8, 33], BF16)
        for j in range(8):
            nc.scalar.dma_start(
                out=v_st[:, j, :], in_=x[j * 128:(j + 1) * 128, 512:768])

        def emit_vconv(j):
            nc.vector.tensor_copy(
                v_aug[:, :, j, 0:32],
                v_st[:, j, :].rearrange("p (h d) -> p h d", h=NH))

        emit_vconv(0)
        nc.vector.tensor_copy(
            v_aug[:, :, :, 32:33].rearrange("p h j o -> p (h j o)"), ones_st)
        emit_vconv(1)
        emit_vconv(2)

        # ---------------- output staging / flush ----------------
        out_sb = persist.tile([128, 8, 256], F32)

        def emit_av2_copy(h, av2t, csl=slice(0, N)):
            s = (h % 2) * 64
            nc.vector.tensor_copy(av2t[0:33, csl], ps_att[s:s + 33, csl])

        def flush_head(h, av2t, nts, tail=False):
            groups = {3: 4, 7: 4}
            for nt in nts:
                csl = slice(nt * 128, (nt + 1) * 128)
                # on the tail, borrow the (by then idle) logits PSUM banks so
                # four transposes can be in flight instead of two
                pool_ = pslog if (tail and nt % 2) else psscr
                tag = "log" if (tail and nt % 2) else "scr"
                ps_t = pool_.tile([128, 33], F32, tag=tag, name=f"ps_t{h}_{nt}")
                nc.tensor.transpose(ps_t, av2t[0:33, csl], ident[0:33, 0:33])
                rec = stagep.tile([128, 1], F32, tag="rec")
                nc.vector.reciprocal(rec, ps_t[:, 32:33])
                nc.vector.tensor_scalar_mul(
                    out_sb[:, nt, h * 32:(h + 1) * 32], ps_t[:, 0:32], rec)
                if nt in groups:
                    group = groups[nt]
                    g = nt - group + 1
                    dstap = bass.AP(
                        tensor=out.tensor,
                        offset=g * 128 * 256 + h * 32,
                        ap=[[256, 128], [128 * 256, group], [1, 32]])
                    nc.sync.dma_start(
                        out=dstap, in_=out_sb[:, g:nt + 1, h * 32:(h + 1) * 32])

        # ---------------- per-head pipeline ----------------
        # early-head hooks: later heads' staging spread across the j-loops
        # so nothing competes with the first heads' critical chains
        HOOKS = {
            (0, 2): lambda a: (emit_vconv(3), emit_vconv(4),
                               emit_transposes(2, qT, 1, 0, pin=True,
                                               pe_anchor=a)),
            (0, 3): lambda a: (emit_bias(1, pin_dve=True, pe_anchor=a),
                               emit_transposes(2, qT, 1, 4, pin=True,
                                               pe_anchor=a)),
            (0, 4): lambda a: (emit_vconv(5), emit_vconv(6),
                               emit_vconv(7), emit_kaug(1), emit_onehot(1)),
            (0, 5): lambda a: (emit_bias(2, pe_anchor=a),
                               emit_transposes(3, kT, 1, 0, pin=True,
                                               pe_anchor=a)),
            (0, 6): lambda a: emit_transposes(3, kT, 1, 4, pin=True,
                                              pe_anchor=a),
            (1, 1): lambda a: (emit_kaug(2), emit_onehot(2)),
            (1, 3): lambda a: (emit_kaug(3), emit_onehot(3), emit_kaug(4)),
            (2, 1): lambda a: emit_kaug(5),
            (3, 1): lambda a: emit_kaug(6),
            (4, 1): lambda a: emit_kaug(7),
        }
        pending = None    # (head, av2t) awaiting transpose+normalize

        def emit_logits(h, j):
            qaug = qaug_all[:, h, :]
            ps_l = pslog.tile([128, N], F32, tag="log", name=f"ps_l{h}_{j}")
            for half in range(2):
                sl = slice(half * 512, (half + 1) * 512)
                nc.tensor.matmul(
                    ps_l[:, sl], lhsT=ka[h % 4][:, j, :],
                    rhs=qaug[:, sl], start=True, stop=True)
            return ps_l

        for h in range(NH):
            s = (h % 2) * 64
            for j in range(8):
                # logits for j are emitted at the end of iteration j-1 so
                # hook fillers can never precede them in the engine queues
                if j == 0:
                    ps_l = emit_logits(h, 0)
                ew = expwp.tile([128, N], BF16, tag="ew", name=f"ew{h}_{j}")
                nc.scalar.activation(ew, ps_l, AF.Exp, scale=SCALE)
                for half in range(2):
                    sl = slice(half * 512, (half + 1) * 512)
                    a_last = nc.tensor.matmul(
                        ps_att[s:s + 33, sl], lhsT=v_aug[:, h, j, :],
                        rhs=ew[:, sl], start=(j == 0), stop=(j == 7))
                if j < 7:
                    ps_l = emit_logits(h, j + 1)
                elif h + 1 < NH:
                    ps_l = None   # next head's j0 emitted at its loop start
                if (h, j) in HOOKS:
                    HOOKS[(h, j)](a_last)
                if j == 2 and pending is not None:
                    flush_head(pending[0], pending[1], range(8))
                    pending = None
                if j == 5 and h >= 1 and h + 2 < NH:
                    emit_bias(h + 2)

            av2t = av2p.tile([33, N], F32, tag="av2", name=f"av2_{h}")
            if h < NH - 1:
                emit_av2_copy(h, av2t)
            else:
                # tail: quarter-split the last copy so the flush transposes
                # start as soon as the first columns land
                for qq in range(4):
                    emit_av2_copy(h, av2t, slice(qq * 256, (qq + 1) * 256))
            pending = (h, av2t)

        # tail flush of the last head
        flush_head(pending[0], pending[1], range(4), tail=True)
        flush_head(pending[0], pending[1], range(4, 8), tail=True)


_NC_CACHE = {}


def _build():
    if "nc" in _NC_CACHE:
        return _NC_CACHE["nc"]
    nc = bass.Bass("TRN2", target_bir_lowering=False, debug=False,
                   enable_asserts=True, num_devices=8)
    ins = {
        "x": nc.dram_tensor("x", [N, 768], F32, kind="ExternalInput").ap(),
        "relh": nc.dram_tensor("relh", [63, 32], F32, kind="ExternalInput").ap(),
        "relw": nc.dram_tensor("relw", [63, 32], F32, kind="ExternalInput").ap(),
    }
    outs = {
        "out": nc.dram_tensor("out", [N, 256], F32, kind="ExternalOutput").ap(),
    }
    with tile.TileContext(nc) as tc:
        kernel_body(tc, outs, ins)
    split_multiwaits(nc)
    _NC_CACHE["nc"] = nc
    return nc


def kernel(inputs, key_rel_h, key_rel_w, _trace=False):
    nc = _build()
    x = np.ascontiguousarray(np.asarray(inputs, dtype=np.float32).reshape(8, N, 768))
    rh = np.ascontiguousarray(np.asarray(key_rel_h, dtype=np.float32))
    rw = np.ascontiguousarray(np.asarray(key_rel_w, dtype=np.float32))
    in_maps = [{"x": x[c], "relh": rh, "relw": rw} for c in range(8)]
    res = bass_utils.run_bass_kernel_spmd(
        nc, in_maps, core_ids=list(range(8)), trace=_trace)
    outp = np.stack([r["out"] for r in res.results])
    if _trace:
        kernel.last_results = res
    return outp.reshape(8, 32, 32, 256)
